# revision 32
# baseline (speedup 1.0000x reference)
"""Trainium2 Bass kernel for nn_Distribution_74758200754679.

Computes, for x [65536, 8, 256] and a tiny MLP (256 -> 128 -> 1):
    h    = leaky_relu(x @ W1 + b1, 0.3)
    beta = sigmoid(h @ W2 + b2)            # [B, N]
    p    = stick_breaking(beta)            # [B, N+1]

Distribution: pure data parallel over 8 NeuronCores — x is sharded along
the batch axis, MLP params are replicated. Each core's shard is staged
host-side in transposed layout (d_in on partitions) so the device loop is
a straight chain of full-rate matmuls with no on-chip transpose.

Per-core device program (64 MB of x per core, 128 blocks x 512 rows):
  DMA xT block chunks -> PE fp32r matmuls (L1, accumulate K=256 in PSUM)
  -> ACT 0.7*relu(z+b1) -> DVE hh = 0.3*z + r (leaky) -> PE L2 matmul
  -> DVE copy [1,512] -> tiny DMA gathers beta rows into [block, row] tile
  -> tail: sigmoid + suffix-product stick-breaking, one contiguous DMA out.
"""

import os
import sys

# The device path runs through jax/PJRT on the neuron (axon) platform; a
# cpu-pinned JAX_PLATFORMS would hide the NeuronCores.
if os.environ.get("JAX_PLATFORMS") == "cpu":
    os.environ["JAX_PLATFORMS"] = ""

for _p in ("/opt/trn_rl_repo",):
    if _p not in sys.path:
        sys.path.insert(0, _p)

import numpy as np
from contextlib import ExitStack

import concourse.bacc as bacc
import concourse.mybir as mybir
from concourse import tile
from concourse import bass_utils

B, N, D_IN, D_H = 65536, 8, 256, 128
SLOPE = 0.3
CORES = 8
RC = B * N // CORES          # rows per core (65536)
BC = B // CORES              # batches per core (8192)
BLK = 512                    # rows per block
NBLK = RC // BLK             # 128
NG = BLK // N                # batch groups per partition in the tail (64)

f32 = mybir.dt.float32
f32r = mybir.dt.float32r
AF = mybir.ActivationFunctionType
ALU = mybir.AluOpType

_NC_CACHE = []
_LAST_RESULTS = None


def _build():
    nc = bacc.Bacc(
        "TRN2", target_bir_lowering=False, debug=False, num_devices=CORES
    )
    xt_d = nc.dram_tensor("xt", [D_IN, RC], f32r, kind="ExternalInput").ap()
    w1_d = nc.dram_tensor("w1", [D_IN, D_H], f32r, kind="ExternalInput").ap()
    w2_d = nc.dram_tensor("w2", [D_H, 1], f32r, kind="ExternalInput").ap()
    bias7_d = nc.dram_tensor("bias7", [D_H, 1], f32, kind="ExternalInput").ap()
    st_d = nc.dram_tensor("st", [128, 1], f32, kind="ExternalInput").ap()
    nst_d = nc.dram_tensor("nst", [128, 1], f32, kind="ExternalInput").ap()
    p_d = nc.dram_tensor("p", [BC, N + 1], f32, kind="ExternalOutput").ap()

    with tile.TileContext(nc) as tc, ExitStack() as ctx:
        const = ctx.enter_context(tc.tile_pool(name="const", bufs=1))
        xpool = ctx.enter_context(tc.tile_pool(name="xp", bufs=1))
        hpool = ctx.enter_context(tc.tile_pool(name="hp", bufs=1))
        bpool = ctx.enter_context(tc.tile_pool(name="bp", bufs=1))
        tpool = ctx.enter_context(tc.tile_pool(name="tp", bufs=1))
        psh = ctx.enter_context(tc.tile_pool(name="psh", bufs=1, space="PSUM"))
        psb = ctx.enter_context(tc.tile_pool(name="psb", bufs=1, space="PSUM"))

        def T(pool, shape, dt_, nm, bufs=1):
            tag = nm.split("_")[0]
            return pool.tile(shape, dt_, name=nm, tag=tag, bufs=bufs)

        GRP = 8                  # compute blocks per DMA / staging group
        DBLK = GRP * BLK         # 4096 cols, 16 KB per partition per chunk

        w1_sb = T(const, [128, 2, D_H], f32r, "w1sb")
        nc.sync.dma_start(w1_sb[:], w1_d.rearrange("(kc p) m -> p kc m", kc=2))
        w2_sb = T(const, [D_H, 1], f32r, "w2sb")
        nc.sync.dma_start(w2_sb[:], w2_d[:])
        bias7_sb = T(const, [D_H, 1], f32, "bias7sb")
        nc.sync.dma_start(bias7_sb[:], bias7_d[:])
        st_sb = T(const, [128, 1], f32, "stsb")
        nc.sync.dma_start(st_sb[:], st_d[:])
        nst_sb = T(const, [128, 1], f32, "nstsb")
        nc.sync.dma_start(nst_sb[:], nst_d[:])

        # beta accumulator: partition = block index, free = row-in-block
        bt = T(bpool, [128, BLK], f32, "bt")

        for dblk in range(NBLK // GRP):
            x0 = T(xpool, [128, DBLK], f32r, f"x0_{dblk}", bufs=4)
            nc.sync.dma_start(x0[:], xt_d[0:128, dblk * DBLK:(dblk + 1) * DBLK])
            x1 = T(xpool, [128, DBLK], f32r, f"x1_{dblk}", bufs=4)
            nc.sync.dma_start(x1[:], xt_d[128:256, dblk * DBLK:(dblk + 1) * DBLK])
            bs = T(bpool, [1, DBLK], f32, f"bs_{dblk}", bufs=3)
            for sub in range(GRP):
                blk = dblk * GRP + sub
                cs = slice(sub * BLK, (sub + 1) * BLK)

                ph = T(psh, [128, BLK], f32, f"ph_{blk}", bufs=4)
                nc.tensor.matmul(ph[:], w1_sb[:, 0, :], x0[:, cs], start=True, stop=False)
                nc.tensor.matmul(ph[:], w1_sb[:, 1, :], x1[:, cs], start=False, stop=True)

                # leaky_relu(z + b1) = 0.3*(z + b1) + 0.7*relu(z + b1)
                #   r  = relu(0.7*z + 0.7*b1)              (ACT)
                #   hh = 0.3*z + r                          (DVE; 0.3*b1 in st)
                r_sb = T(hpool, [128, BLK], f32, f"r_{blk}", bufs=4)
                nc.scalar.activation(
                    r_sb[:], ph[:], AF.Relu, bias=bias7_sb[:], scale=0.7
                )
                hh = T(hpool, [128, BLK], f32r, f"hh_{blk}", bufs=4)
                nc.vector.scalar_tensor_tensor(
                    hh[:], ph[:], SLOPE, r_sb[:], op0=ALU.mult, op1=ALU.add
                )

                pb = T(psb, [1, BLK], f32, f"pb_{blk}", bufs=4)
                nc.tensor.matmul(pb[:], w2_sb[:], hh[:], start=True, stop=True)
                # PSUM -> SBUF staging of beta_pre rows: 1-lane copies,
                # split between DVE and ACT so neither chokes.
                if blk % 2 == 0:
                    nc.vector.tensor_copy(bs[0:1, cs], pb[:])
                else:
                    nc.scalar.activation(bs[0:1, cs], pb[:], AF.Copy)
            # one fan-out DMA redistributes GRP beta rows to partition-per-block
            nc.scalar.dma_start(
                bt[dblk * GRP:(dblk + 1) * GRP, :],
                bs[:].rearrange("p (j r) -> p j r", j=GRP),
            )

        # ---- tail: stick-breaking over the N axis (groups of 8 in free dim)
        sg = T(tpool, [128, BLK], f32, "sg")
        nc.scalar.activation(sg[:], bt[:], AF.Sigmoid, bias=st_sb[:], scale=1.0)
        g = T(tpool, [128, BLK], f32, "g")  # 1 - beta = sigmoid(-(x + st))
        nc.scalar.activation(g[:], bt[:], AF.Sigmoid, bias=nst_sb[:], scale=-1.0)

        # suffix products s[e] = prod_{k>=e} g[k] via in-place log-tree:
        # s[0:N-k] *= s[k:N] reads ahead of writes (forward refs are safe)
        s = T(tpool, [128, BLK], f32, "s")
        nc.vector.tensor_copy(s[:], g[:])
        sv = s[:].rearrange("p (gr e) -> p gr e", e=N)
        for k in (1, 2, 4):
            nc.vector.tensor_mul(sv[:, :, 0:N - k], sv[:, :, 0:N - k], sv[:, :, k:N])

        # P[gr*9]     = s[gr*8]                   (p[b, 0])
        # P[gr*9 + i] = beta[i-1] * s[i], i=1..7  (s[8] == 1 -> P[..,8]=beta[7])
        P = T(tpool, [128, NG * (N + 1)], f32, "P")
        Pv = P[:].rearrange("p (gr e) -> p gr e", e=N + 1)
        sgv = sg[:].rearrange("p (gr e) -> p gr e", e=N)
        nc.vector.tensor_copy(Pv[:, :, 0:1], sv[:, :, 0:1])
        nc.vector.tensor_mul(Pv[:, :, 1:N], sgv[:, :, 0:N - 1], sv[:, :, 1:N])
        nc.vector.tensor_copy(Pv[:, :, N:N + 1], sgv[:, :, N - 1:N])
        nc.sync.dma_start(
            p_d.rearrange("(blk gr) e -> blk (gr e)", gr=NG), P[:]
        )

    nc.compile()
    return nc


def _get_nc():
    if not _NC_CACHE:
        _NC_CACHE.append(_build())
    return _NC_CACHE[0]


def kernel(**inputs):
    x = np.asarray(inputs["x"], dtype=np.float32)
    W1 = np.ascontiguousarray(np.asarray(inputs["W1"], dtype=np.float32))
    b1 = np.asarray(inputs["b1"], dtype=np.float32)
    W2 = np.ascontiguousarray(np.asarray(inputs["W2"], dtype=np.float32))
    b2 = np.asarray(inputs["b2"], dtype=np.float32)

    nc = _get_nc()

    xf = x.reshape(B * N, D_IN)
    st_val = np.float32(float(b2[0]) + SLOPE * float(b1 @ W2[:, 0]))
    bias7 = np.ascontiguousarray((0.7 * b1).reshape(D_H, 1).astype(np.float32))
    stv = np.full((128, 1), st_val, np.float32)
    nstv = np.ascontiguousarray(-stv)

    in_maps = []
    for c in range(CORES):
        shard = xf[c * RC:(c + 1) * RC]
        xt = np.ascontiguousarray(shard.T)   # [256, RC]
        in_maps.append({
            "xt": xt, "w1": W1, "w2": W2,
            "bias7": bias7, "st": stv, "nst": nstv,
        })

    res = bass_utils.run_bass_kernel_spmd(
        nc, in_maps, core_ids=list(range(CORES))
    )
    global _LAST_RESULTS
    _LAST_RESULTS = res
    p = np.concatenate(
        [res.results[c]["p"] for c in range(CORES)], axis=0
    ).astype(np.float32)
    return p
